# revision 4
# baseline (speedup 1.0000x reference)
"""HashEmbedder (instant-NGP style) lookup kernel for 8 NeuronCores.

Contract: kernel(**inputs) takes FULL inputs (x: [2097152, 3] f32,
tables: [5217937, 2] f32) and returns the FULL output [2097152, 32] f32.

Strategy: data-parallel over the point dimension across the 8
axon-tunneled NeuronCores via jax shard_map; hash tables replicated on
every device.  The per-level computation (scale, floor, trilinear
weights, dense linear index or instant-NGP spatial hash, table gather,
weighted reduction) is compiled by neuronx-cc per level.  Each level's
points are processed in sub-batches of 32768 points/core so the
neuronx-cc gather lowering stays within its working size (full-shard
gathers crash walrus).  Compiled NEFFs are disk-cached
(~/.neuron-compile-cache), so repeat invocations skip compilation.
"""

import sys

import numpy as np

sys.path.insert(0, "/opt/trn_rl_repo")

import jax
import jax.numpy as jnp

N_POINTS = 2_097_152
N_CORES = 8
N_LEVELS = 16
F = 2
LOG2_HASHMAP = 19
HASH_SIZE = 1 << LOG2_HASHMAP
HASH_MASK = HASH_SIZE - 1
BASE_RES = 16.0
FINEST_RES = 512.0
P2 = np.uint32(2654435761)
P3 = np.uint32(805459861)
CORNERS = np.array(
    [[i, j, k] for i in (0, 1) for j in (0, 1) for k in (0, 1)], dtype=np.int32
)

SUB = 32768  # points per core per dispatch (keeps neuronx-cc gather happy)


def _level_meta():
    b = np.exp(
        (np.log(np.float32(FINEST_RES)) - np.log(np.float32(BASE_RES)))
        / np.float32(N_LEVELS - 1)
    ).astype(np.float32)
    res = np.floor(
        np.float32(BASE_RES) * (b ** np.arange(N_LEVELS, dtype=np.float32))
    ).astype(np.int64)
    sizes = [min(HASH_SIZE, int(r) ** 3) for r in res]
    offsets = np.concatenate([[0], np.cumsum(sizes)]).astype(np.int64)
    return res, sizes, offsets


_RES, _SIZES, _OFFSETS = _level_meta()


def _level_fn(lvl):
    """Per-level computation: x [n,3] f32, tables [rows,2] -> [n,2]."""
    r = int(_RES[lvl])
    base = int(_OFFSETS[lvl])
    dense = r**3 <= HASH_SIZE
    corners = jnp.asarray(CORNERS)

    def f(x, tables):
        rf = jnp.float32(r)
        xc = jnp.clip(x, 0.0, 1.0)
        off_coords = xc * rf + 0.5
        bl = jnp.floor(off_coords).astype(jnp.int32)
        fracs = off_coords - bl.astype(x.dtype)
        vox = bl[:, None, :] + corners[None]
        if dense:
            w = vox % r
            idx = w[..., 0] + w[..., 1] * r + w[..., 2] * (r * r)
        else:
            v = vox.astype(jnp.uint32)
            h = (v[..., 0] * jnp.uint32(1)) ^ (v[..., 1] * P2) ^ (v[..., 2] * P3)
            idx = (h & jnp.uint32(HASH_MASK)).astype(jnp.int32)
        emb = tables[base + idx]  # [n,8,2]
        wts = jnp.where(
            corners[None] == 0, 1.0 - fracs[:, None, :], fracs[:, None, :]
        ).prod(-1)
        return jnp.einsum("nc,ncf->nf", wts, emb)

    return f


_state = None


def _get_state():
    global _state
    if _state is None:
        from jax.sharding import Mesh, PartitionSpec as P

        try:
            from jax import shard_map
        except ImportError:
            from jax.experimental.shard_map import shard_map

        devices = jax.devices()[:N_CORES]
        mesh = Mesh(np.asarray(devices), ("core",))
        jits = []
        for lvl in range(N_LEVELS):
            f = _level_fn(lvl)
            try:
                sf = shard_map(
                    f,
                    mesh=mesh,
                    in_specs=(P("core"), P()),  # x sharded, tables replicated
                    out_specs=P("core"),
                    check_vma=False,
                )
            except TypeError:
                sf = shard_map(
                    f,
                    mesh=mesh,
                    in_specs=(P("core"), P()),
                    out_specs=P("core"),
                    check_rep=False,
                )
            jits.append(jax.jit(sf))
        _state = (jits, mesh)
    return _state


def kernel(x: np.ndarray, tables: np.ndarray) -> np.ndarray:
    x = np.ascontiguousarray(np.asarray(x, dtype=np.float32))
    tables = np.ascontiguousarray(np.asarray(tables, dtype=np.float32))
    n = x.shape[0]
    shard = n // N_CORES

    jits, mesh = _get_state()
    from jax.sharding import NamedSharding, PartitionSpec as P

    xsh = NamedSharding(mesh, P("core"))
    tabs = jax.device_put(tables, NamedSharding(mesh, P()))

    # Reorder so each dispatch of SUB*N_CORES points is sharded contiguously:
    # global dispatch slice s covers, on core c, points c*shard + s*SUB ...
    x3 = x.reshape(N_CORES, shard, 3)
    n_sub = shard // SUB

    # device_put each dispatch slice: [N_CORES*SUB, 3] with core-major layout
    outs = [[None] * N_LEVELS for _ in range(n_sub)]
    xs_dev = []
    for s in range(n_sub):
        xs = x3[:, s * SUB : (s + 1) * SUB, :].reshape(N_CORES * SUB, 3)
        xs_dev.append(jax.device_put(xs, xsh))
    for s in range(n_sub):
        for lvl in range(N_LEVELS):
            outs[s][lvl] = jits[lvl](xs_dev[s], tabs)

    out = np.empty((N_CORES, shard, N_LEVELS * F), dtype=np.float32)
    for s in range(n_sub):
        for lvl in range(N_LEVELS):
            o = np.asarray(outs[s][lvl]).reshape(N_CORES, SUB, F)
            out[:, s * SUB : (s + 1) * SUB, lvl * F : (lvl + 1) * F] = o
    return out.reshape(n, N_LEVELS * F)


if __name__ == "__main__":
    import time

    rng = np.random.default_rng(0)
    x = rng.random((N_POINTS, 3), dtype=np.float32)
    tables = ((rng.random((int(_OFFSETS[-1]), F), dtype=np.float32)) - 0.5) * 2e-4
    t0 = time.time()
    out = kernel(x, tables)
    print("kernel time:", time.time() - t0, flush=True)
    t0 = time.time()
    out = kernel(x, tables)
    print("second call:", time.time() - t0, flush=True)


# revision 8
# speedup vs baseline: 1.6127x; 1.6127x over previous
"""HashEmbedder (instant-NGP style) lookup kernel for 8 NeuronCores.

Contract: kernel(**inputs) takes FULL inputs (x: [2097152, 3] f32,
tables: [5217937, 2] f32) and returns the FULL output [2097152, 32] f32.

Strategy: data-parallel over the point dimension across the 8
axon-tunneled NeuronCores via jax shard_map; hash tables replicated on
every device.  All 16 levels (scale, floor, trilinear weights, dense
linear index or instant-NGP spatial hash, table gather, weighted
reduction) are fused into one jitted program that processes SUB points
per core per dispatch — SUB is capped so each level's gather stays
within the size neuronx-cc's gather lowering handles (full-shard
gathers crash walrus).  Compiled NEFFs are disk-cached
(~/.neuron-compile-cache), so repeat invocations skip compilation.
"""

import sys

import numpy as np

sys.path.insert(0, "/opt/trn_rl_repo")

import jax
import jax.numpy as jnp

N_POINTS = 2_097_152
N_CORES = 8
N_LEVELS = 16
F = 2
LOG2_HASHMAP = 19
HASH_SIZE = 1 << LOG2_HASHMAP
HASH_MASK = HASH_SIZE - 1
BASE_RES = 16.0
FINEST_RES = 512.0
P2 = np.uint32(2654435761)
P3 = np.uint32(805459861)
CORNERS = np.array(
    [[i, j, k] for i in (0, 1) for j in (0, 1) for k in (0, 1)], dtype=np.int32
)

SUB = 8192  # points per core per dispatch


def _level_meta():
    b = np.exp(
        (np.log(np.float32(FINEST_RES)) - np.log(np.float32(BASE_RES)))
        / np.float32(N_LEVELS - 1)
    ).astype(np.float32)
    res = np.floor(
        np.float32(BASE_RES) * (b ** np.arange(N_LEVELS, dtype=np.float32))
    ).astype(np.int64)
    sizes = [min(HASH_SIZE, int(r) ** 3) for r in res]
    offsets = np.concatenate([[0], np.cumsum(sizes)]).astype(np.int64)
    return res, sizes, offsets


_RES, _SIZES, _OFFSETS = _level_meta()


def _level_fn(lvl):
    """Per-level computation: x [n,3] f32, tables [rows,2] -> [n,2]."""
    r = int(_RES[lvl])
    base = int(_OFFSETS[lvl])
    dense = r**3 <= HASH_SIZE
    corners = jnp.asarray(CORNERS)

    def f(x, tables):
        rf = jnp.float32(r)
        xc = jnp.clip(x, 0.0, 1.0)
        off_coords = xc * rf + 0.5
        bl = jnp.floor(off_coords).astype(jnp.int32)
        fracs = off_coords - bl.astype(x.dtype)
        vox = bl[:, None, :] + corners[None]
        if dense:
            w = vox % r
            idx = w[..., 0] + w[..., 1] * r + w[..., 2] * (r * r)
        else:
            v = vox.astype(jnp.uint32)
            h = (v[..., 0] * jnp.uint32(1)) ^ (v[..., 1] * P2) ^ (v[..., 2] * P3)
            idx = (h & jnp.uint32(HASH_MASK)).astype(jnp.int32)
        emb = tables[base + idx]  # [n,8,2]
        wts = jnp.where(
            corners[None] == 0, 1.0 - fracs[:, None, :], fracs[:, None, :]
        ).prod(-1)
        return jnp.einsum("nc,ncf->nf", wts, emb)

    return f


def _fused(x, tables):
    return jnp.concatenate([_level_fn(l)(x, tables) for l in range(N_LEVELS)], -1)


_state = None


def _get_state():
    global _state
    if _state is None:
        from jax.sharding import Mesh, PartitionSpec as P

        try:
            from jax import shard_map
        except ImportError:
            from jax.experimental.shard_map import shard_map

        devices = jax.devices()[:N_CORES]
        mesh = Mesh(np.asarray(devices), ("core",))
        try:
            sf = shard_map(
                _fused,
                mesh=mesh,
                in_specs=(P("core"), P()),  # x sharded, tables replicated
                out_specs=P("core"),
                check_vma=False,
            )
        except TypeError:
            sf = shard_map(
                _fused,
                mesh=mesh,
                in_specs=(P("core"), P()),
                out_specs=P("core"),
                check_rep=False,
            )
        _state = (jax.jit(sf), mesh)
    return _state


def kernel(x: np.ndarray, tables: np.ndarray) -> np.ndarray:
    x = np.ascontiguousarray(np.asarray(x, dtype=np.float32))
    tables = np.ascontiguousarray(np.asarray(tables, dtype=np.float32))
    n = x.shape[0]
    shard = n // N_CORES

    jf, mesh = _get_state()
    from jax.sharding import NamedSharding, PartitionSpec as P

    xsh = NamedSharding(mesh, P("core"))
    tabs = jax.device_put(tables, NamedSharding(mesh, P()))

    # Each dispatch handles SUB points per core; core c's points are the
    # contiguous run c*shard ... (c+1)*shard, sliced into n_sub SUB-blocks.
    x3 = x.reshape(N_CORES, shard, 3)
    n_sub = shard // SUB

    merged = []
    for s in range(n_sub):
        xs = x3[:, s * SUB : (s + 1) * SUB, :].reshape(N_CORES * SUB, 3)
        merged.append(jf(jax.device_put(xs, xsh), tabs))

    out = np.empty((N_CORES, shard, N_LEVELS * F), dtype=np.float32)
    for s in range(n_sub):
        o = np.asarray(merged[s]).reshape(N_CORES, SUB, N_LEVELS * F)
        out[:, s * SUB : (s + 1) * SUB, :] = o
    return out.reshape(n, N_LEVELS * F)


if __name__ == "__main__":
    import time

    rng = np.random.default_rng(0)
    x = rng.random((N_POINTS, 3), dtype=np.float32)
    tables = ((rng.random((int(_OFFSETS[-1]), F), dtype=np.float32)) - 0.5) * 2e-4
    t0 = time.time()
    out = kernel(x, tables)
    print("kernel time:", time.time() - t0, flush=True)
    t0 = time.time()
    out = kernel(x, tables)
    print("second call:", time.time() - t0, flush=True)


# revision 9
# speedup vs baseline: 1.8779x; 1.1644x over previous
"""HashEmbedder (instant-NGP style) lookup kernel for 8 NeuronCores.

Contract: kernel(**inputs) takes FULL inputs (x: [2097152, 3] f32,
tables: [5217937, 2] f32) and returns the FULL output [2097152, 32] f32.

Strategy: data-parallel over the point dimension across the 8
axon-tunneled NeuronCores via jax shard_map; hash tables replicated on
every device.  All 16 levels (scale, floor, trilinear weights, dense
linear index or instant-NGP spatial hash, table gather, weighted
reduction) are fused into one jitted program that processes SUB points
per core per dispatch — SUB is capped so each level's gather stays
within the size neuronx-cc's gather lowering handles (full-shard
gathers crash walrus).  Compiled NEFFs are disk-cached
(~/.neuron-compile-cache), so repeat invocations skip compilation.
"""

import sys

import numpy as np

sys.path.insert(0, "/opt/trn_rl_repo")

import jax
import jax.numpy as jnp

N_POINTS = 2_097_152
N_CORES = 8
N_LEVELS = 16
F = 2
LOG2_HASHMAP = 19
HASH_SIZE = 1 << LOG2_HASHMAP
HASH_MASK = HASH_SIZE - 1
BASE_RES = 16.0
FINEST_RES = 512.0
P2 = np.uint32(2654435761)
P3 = np.uint32(805459861)
CORNERS = np.array(
    [[i, j, k] for i in (0, 1) for j in (0, 1) for k in (0, 1)], dtype=np.int32
)

SUB = 8192  # points per core per dispatch


def _level_meta():
    b = np.exp(
        (np.log(np.float32(FINEST_RES)) - np.log(np.float32(BASE_RES)))
        / np.float32(N_LEVELS - 1)
    ).astype(np.float32)
    res = np.floor(
        np.float32(BASE_RES) * (b ** np.arange(N_LEVELS, dtype=np.float32))
    ).astype(np.int64)
    sizes = [min(HASH_SIZE, int(r) ** 3) for r in res]
    offsets = np.concatenate([[0], np.cumsum(sizes)]).astype(np.int64)
    return res, sizes, offsets


_RES, _SIZES, _OFFSETS = _level_meta()


def _level_fn(lvl):
    """Per-level computation: x [n,3] f32, tables [rows,2] -> [n,2]."""
    r = int(_RES[lvl])
    base = int(_OFFSETS[lvl])
    dense = r**3 <= HASH_SIZE
    corners = jnp.asarray(CORNERS)

    def f(x, tables):
        rf = jnp.float32(r)
        xc = jnp.clip(x, 0.0, 1.0)
        off_coords = xc * rf + 0.5
        bl = jnp.floor(off_coords).astype(jnp.int32)
        fracs = off_coords - bl.astype(x.dtype)
        vox = bl[:, None, :] + corners[None]
        if dense:
            w = vox % r
            idx = w[..., 0] + w[..., 1] * r + w[..., 2] * (r * r)
        else:
            v = vox.astype(jnp.uint32)
            h = (v[..., 0] * jnp.uint32(1)) ^ (v[..., 1] * P2) ^ (v[..., 2] * P3)
            idx = (h & jnp.uint32(HASH_MASK)).astype(jnp.int32)
        emb = tables[base + idx]  # [n,8,2]
        wts = jnp.where(
            corners[None] == 0, 1.0 - fracs[:, None, :], fracs[:, None, :]
        ).prod(-1)
        return jnp.einsum("nc,ncf->nf", wts, emb)

    return f


def _fused(x, tables):
    return jnp.concatenate([_level_fn(l)(x, tables) for l in range(N_LEVELS)], -1)


_state = None


def _get_state():
    global _state
    if _state is None:
        from jax.sharding import Mesh, PartitionSpec as P

        try:
            from jax import shard_map
        except ImportError:
            from jax.experimental.shard_map import shard_map

        devices = jax.devices()[:N_CORES]
        mesh = Mesh(np.asarray(devices), ("core",))
        try:
            sf = shard_map(
                _fused,
                mesh=mesh,
                in_specs=(P("core"), P()),  # x sharded, tables replicated
                out_specs=P("core"),
                check_vma=False,
            )
        except TypeError:
            sf = shard_map(
                _fused,
                mesh=mesh,
                in_specs=(P("core"), P()),
                out_specs=P("core"),
                check_rep=False,
            )
        _state = (jax.jit(sf), mesh)
    return _state


def kernel(x: np.ndarray, tables: np.ndarray) -> np.ndarray:
    x = np.ascontiguousarray(np.asarray(x, dtype=np.float32))
    tables = np.ascontiguousarray(np.asarray(tables, dtype=np.float32))
    n = x.shape[0]
    shard = n // N_CORES

    jf, mesh = _get_state()
    from jax.sharding import NamedSharding, PartitionSpec as P

    xsh = NamedSharding(mesh, P("core"))
    tabs = jax.device_put(tables, NamedSharding(mesh, P()))

    # Each dispatch handles SUB points per core; core c's points are the
    # contiguous run c*shard ... (c+1)*shard, sliced into n_sub SUB-blocks.
    x3 = x.reshape(N_CORES, shard, 3)
    n_sub = shard // SUB

    merged = []
    for s in range(n_sub):
        xs = x3[:, s * SUB : (s + 1) * SUB, :].reshape(N_CORES * SUB, 3)
        merged.append(jf(jax.device_put(xs, xsh), tabs))

    out = np.empty((N_CORES, shard, N_LEVELS * F), dtype=np.float32)
    try:
        # fetch everything in one big D2H transfer (many small axon
        # transfers are far slower than one large one)
        big = np.asarray(jnp.concatenate(merged, axis=0))
        big = big.reshape(n_sub, N_CORES, SUB, N_LEVELS * F)
        for s in range(n_sub):
            out[:, s * SUB : (s + 1) * SUB, :] = big[s]
    except Exception:
        for s in range(n_sub):
            o = np.asarray(merged[s]).reshape(N_CORES, SUB, N_LEVELS * F)
            out[:, s * SUB : (s + 1) * SUB, :] = o
    return out.reshape(n, N_LEVELS * F)


if __name__ == "__main__":
    import time

    rng = np.random.default_rng(0)
    x = rng.random((N_POINTS, 3), dtype=np.float32)
    tables = ((rng.random((int(_OFFSETS[-1]), F), dtype=np.float32)) - 0.5) * 2e-4
    t0 = time.time()
    out = kernel(x, tables)
    print("kernel time:", time.time() - t0, flush=True)
    t0 = time.time()
    out = kernel(x, tables)
    print("second call:", time.time() - t0, flush=True)
